# revision 6
# baseline (speedup 1.0000x reference)
"""Trainium2 Bass kernel for nn_DTRN: 2-layer bidirectional discount-gated LSTM
with a bidirectional-LSTM discount-coefficient generator.

Sharding: data-parallel over batch, B=16 -> 2 per core across 8 cores (SPMD,
no collectives). Per core, three sequential scan phases (d -> l0 -> l1), each
running the forward and backward time scans interleaved.

Layouts (per core, BL = local batch = 2):
  - Gates in PSUM chunk-layout [128, 8*BL]: 4H=1024 split into 8 chunks of 128
    rows; host permutes gate order to (i,f,o,g) so one sigmoid covers a
    contiguous 6*BL block and tanh the trailing 2*BL block.
  - Recurrent matmul is weight-stationary bf16: 16 [128x128]x[128,BL] matmuls
    per step accumulating onto the precomputed input projection (injected into
    PSUM by an identity matmul).
  - Input projections computed in bulk per 64-step block from a transposed
    SBUF-resident copy of the inputs; bias folded in during the PSUM->SBUF
    copy on ScalarE (which also casts to bf16).
  - h histories stay in SBUF transposed ([128, k, T, BL] bf16): they feed the
    next step's recurrent matmul, the next layer's bulk projections, the
    coefficient projections, and the final PE-transpose output path.
  - Discount coefficients are computed in bulk between phases (matvec +
    sigmoid), then broadcast across partitions per block via a rank-1 matmul
    against a ones vector.
"""

import sys

sys.path.insert(0, "/opt/trn_rl_repo")

from contextlib import ExitStack

import numpy as np
import ml_dtypes

import concourse.bass as bass
import concourse.tile as tile
from concourse import bacc, mybir
from concourse import bass_utils

F32 = mybir.dt.float32
BF16 = mybir.dt.bfloat16
AF = mybir.ActivationFunctionType
DF = 0.9

B, T_FULL, I, H = 16, 2048, 256, 256
NCORES = 8
BL = B // NCORES  # local batch per core
G4 = 4 * H  # 1024
NCH = G4 // 128  # 8 gate chunks
S = 64  # scan steps per block

# gate-permutation: reorder (i,f,g,o) -> (i,f,o,g) so the sigmoid block is
# contiguous (chunks 0..5) and tanh covers chunks 6..7
_PERM = np.r_[0:2 * H, 3 * H:4 * H, 2 * H:3 * H]


def _ds(e, n):
    return bass.ds(e, n)


class _Builder:
    def __init__(self, T, use_for_i=True, debug_outs=False):
        assert T % S == 0
        self.T = T
        self.use_for_i = use_for_i
        self.debug_outs = debug_outs
        self.nc = bacc.Bacc("TRN2", target_bir_lowering=False, debug=False)

    # ---------------- DRAM I/O ----------------
    def declare_io(self):
        nc = self.nc
        T = self.T
        self.d_xT = nc.dram_tensor("xT", [2 * 128, T, BL], BF16, kind="ExternalInput").ap()
        self.d_w = {}
        for ph, nk in (("d", 2), ("l0", 2), ("l1", 4)):
            for dr in ("f", "b"):
                self.d_w[f"{ph}wih_{dr}"] = nc.dram_tensor(
                    f"{ph}wih_{dr}", [nk * 128, G4], BF16, kind="ExternalInput").ap()
                self.d_w[f"{ph}whh_{dr}"] = nc.dram_tensor(
                    f"{ph}whh_{dr}", [2 * 128, G4], BF16, kind="ExternalInput").ap()
                self.d_w[f"{ph}bias_{dr}"] = nc.dram_tensor(
                    f"{ph}bias_{dr}", [128, NCH], F32, kind="ExternalInput").ap()
        for dr in ("f", "b"):
            self.d_w[f"cwT_{dr}"] = nc.dram_tensor(
                f"cwT_{dr}", [2 * 128, 1], BF16, kind="ExternalInput").ap()
            self.d_w[f"cb_{dr}"] = nc.dram_tensor(
                f"cb_{dr}", [1, 1], F32, kind="ExternalInput").ap()
        self.d_ident = nc.dram_tensor("ident", [128, 128], BF16, kind="ExternalInput").ap()
        self.d_ones = nc.dram_tensor("ones", [1, 128], BF16, kind="ExternalInput").ap()
        self.d_out = nc.dram_tensor("out", [BL, T, 2 * H], F32, kind="ExternalOutput").ap()
        self.d_dbg = {}
        if self.debug_outs:
            for nm in ("hist_d_f", "hist_d_b", "hist_l0_f", "hist_l0_b"):
                self.d_dbg[nm] = nc.dram_tensor(
                    "dbg_" + nm, [128, 2, self.T, BL], BF16, kind="ExternalOutput").ap()
            for nm in ("cf_f", "cf_b"):
                self.d_dbg[nm] = nc.dram_tensor(
                    "dbg_" + nm, [1, self.T, BL], BF16, kind="ExternalOutput").ap()

    # ---------------- build ----------------
    def build(self, ctx: ExitStack, tc: tile.TileContext):
        nc = self.nc
        T = self.T
        self.tc = tc

        persist = ctx.enter_context(tc.tile_pool(name="persist", bufs=1))
        wpool = ctx.enter_context(tc.tile_pool(name="weights", bufs=1))
        self.psum_g = ctx.enter_context(tc.tile_pool(name="psum_g", bufs=4, space="PSUM"))
        self.psum_x = ctx.enter_context(tc.tile_pool(name="psum_x", bufs=2, space="PSUM"))
        self.psum_c = ctx.enter_context(tc.tile_pool(name="psum_c", bufs=2, space="PSUM"))
        self.xp_pool = ctx.enter_context(tc.tile_pool(name="xp_stage", bufs=2))
        self.cfb_pool = ctx.enter_context(tc.tile_pool(name="cfb", bufs=2))
        self.gp = ctx.enter_context(tc.tile_pool(name="gates", bufs=4))
        self.outp = ctx.enter_context(tc.tile_pool(name="outstage", bufs=4))

        # --- constants & inputs resident in SBUF ---
        self.ident = persist.tile([128, 128], BF16, tag="ident", name="ident")
        nc.sync.dma_start(self.ident[:], self.d_ident)
        self.ones = persist.tile([1, 128], BF16, tag="ones", name="ones")
        nc.sync.dma_start(self.ones[:], self.d_ones)
        self.zrow = persist.tile([128, 2, BL], BF16, tag="zrow", name="zrow")
        nc.vector.memset(self.zrow[:], 0.0)

        self.xT = persist.tile([128, 2, T, BL], BF16, tag="xT", name="xT")
        for k in range(2):
            nc.sync.dma_start(self.xT[:, k], self.d_xT[k * 128:(k + 1) * 128])

        # histories
        self.hist = {}
        for ph in ("d", "l0", "l1"):
            for dr in ("f", "b"):
                self.hist[(ph, dr)] = persist.tile(
                    [128, 2, T, BL], BF16, tag=f"hist_{ph}_{dr}", name=f"hist_{ph}_{dr}")

        # coefficient buffers (written after d phase)
        self.cf = {dr: persist.tile([1, T, BL], BF16, tag=f"cf_{dr}", name=f"cf_{dr}")
                   for dr in ("f", "b")}

        # c state
        self.c_sb = {dr: persist.tile([128, 2, BL], F32, tag=f"c_{dr}", name=f"c_{dr}")
                     for dr in ("f", "b")}

        # coefficient weights
        self.cwT = {}
        self.cb = {}
        for dr in ("f", "b"):
            t = persist.tile([128, 2, 1], BF16, tag=f"cwT_{dr}", name=f"cwT_{dr}")
            for k in range(2):
                nc.sync.dma_start(t[:, k], self.d_w[f"cwT_{dr}"][k * 128:(k + 1) * 128])
            self.cwT[dr] = t
            tb = persist.tile([1, 1], F32, tag=f"cb_{dr}", name=f"cb_{dr}")
            nc.sync.dma_start(tb[:], self.d_w[f"cb_{dr}"])
            self.cb[dr] = tb

        # --- phases ---
        for ph, nk in (("d", 2), ("l0", 2), ("l1", 4)):
            wih, whh, bias = {}, {}, {}
            for dr in ("f", "b"):
                w1 = wpool.tile([128, 4, G4], BF16, tag=f"wih_{dr}", name=f"{ph}wih_{dr}")
                for k in range(nk):
                    nc.sync.dma_start(w1[:, k], self.d_w[f"{ph}wih_{dr}"][k * 128:(k + 1) * 128])
                wih[dr] = w1
                w2 = wpool.tile([128, 2, G4], BF16, tag=f"whh_{dr}", name=f"{ph}whh_{dr}")
                for k in range(2):
                    nc.sync.dma_start(w2[:, k], self.d_w[f"{ph}whh_{dr}"][k * 128:(k + 1) * 128])
                whh[dr] = w2
                bt = wpool.tile([128, NCH], F32, tag=f"bias_{dr}", name=f"{ph}bias_{dr}")
                nc.sync.dma_start(bt[:], self.d_w[f"{ph}bias_{dr}"])
                bias[dr] = bt
                nc.vector.memset(self.c_sb[dr][:], 0.0)
            self.phase(ph, nk, wih, whh, bias)
            if ph == "d":
                self.coeff_bulk()

        self.write_out()
        for nm, ap in self.d_dbg.items():
            if nm.startswith("hist"):
                _, p2, d2 = nm.split("_")
                nc.sync.dma_start(ap, self.hist[(p2, d2)][:])
            else:
                dr = nm.split("_")[1]
                nc.sync.dma_start(ap, self.cf[dr][:])

    # ---------------- xp source ----------------
    def xp_rhs(self, ph, kc, texpr, n):
        """rhs [128, n, BL] (t-ascending) for bulk input projection, chunk kc."""
        if ph in ("d", "l0"):
            return self.xT[:, kc, _ds(texpr, n), :]
        src = self.hist[("l0", "f")] if kc < 2 else self.hist[("l0", "b")]
        return src[:, kc % 2, _ds(texpr, n), :]

    # ---------------- one phase ----------------
    def phase(self, ph, nk, wih, whh, bias):
        nblk = self.T // S

        self.block(ph, nk, wih, whh, bias, 0, True)
        if nblk > 1:
            if self.use_for_i:
                with self.tc.For_i(1, nblk, 1) as i:
                    self.block(ph, nk, wih, whh, bias, i, False)
            else:
                for i in range(1, nblk):
                    self.block(ph, nk, wih, whh, bias, i, False)

    # ---------------- one 64-step block ----------------
    def block(self, ph, nk, wih, whh, bias, i, first):
        nc = self.nc
        T = self.T
        t0f = i * S            # forward block start (ascending)
        t0b = (T - S) - i * S  # backward block covers [t0b, t0b+S), consumed descending

        gated = ph != "d"
        xp = {}
        cfb = {}
        for dr, t0 in (("f", t0f), ("b", t0b)):
            # ---- bulk input projection for this block ----
            stage = self.xp_pool.tile([128, S, NCH, BL], BF16, tag=f"xp_{dr}",
                                      name=f"xp_{dr}")
            for m in range(NCH):
                px = self.psum_x.tile([128, S * BL], F32, tag="px", name="px")
                for kc in range(nk):
                    nc.tensor.matmul(
                        px[:],
                        lhsT=wih[dr][:, kc, m * 128:(m + 1) * 128],
                        rhs=self.xp_rhs(ph, kc, t0, S),
                        start=(kc == 0),
                        stop=(kc == nk - 1),
                    )
                # psum -> staging with bias add + bf16 cast (ScalarE)
                nc.scalar.activation(
                    stage[:, :, m, :],
                    px[:],
                    AF.Identity,
                    bias=bias[dr][:, m:m + 1],
                )
            xp[dr] = stage
            if gated:
                # ---- broadcast coefficients across partitions for this block ----
                pc = self.psum_c.tile([128, S * BL], F32, tag="pc", name="pc")
                nc.tensor.matmul(
                    pc[:],
                    lhsT=self.ones[:],
                    rhs=self.cf[dr][:, _ds(t0, S), :],
                    start=True,
                    stop=True,
                )
                cfv = self.cfb_pool.tile([128, S, BL], BF16, tag=f"cfb_{dr}",
                                         name=f"cfb_{dr}")
                nc.vector.tensor_copy(cfv[:], pc[:])
                cfb[dr] = cfv

        # ---- the 64 scan steps ----
        for s in range(S):
            for dr in ("f", "b"):
                if dr == "f":
                    sx = s
                    texpr = t0f + s
                else:
                    sx = S - 1 - s
                    texpr = t0b + sx
                hist = self.hist[(ph, dr)]
                at_start = first and s == 0
                if at_start:
                    hprev = self.zrow[:]
                else:
                    tprev = (t0f + s - 1) if dr == "f" else (t0b + sx + 1)
                    hprev = hist[:, :, _ds(tprev, 1), :]

                g = self.psum_g.tile([128, NCH * BL], F32, tag="g", name="g")
                nc.tensor.matmul(
                    g[:], lhsT=self.ident[:],
                    rhs=xp[dr][:, sx],
                    start=True, stop=False, skip_group_check=True,
                )
                for kc in range(2):
                    hk = self.zrow[:, kc] if at_start else hprev[:, kc]
                    for m in range(NCH):
                        nc.tensor.matmul(
                            g[:, m * BL:(m + 1) * BL],
                            lhsT=whh[dr][:, kc, m * 128:(m + 1) * 128],
                            rhs=hk,
                            start=False, stop=(kc == 1 and m == NCH - 1),
                            skip_group_check=True,
                        )

                # gates
                sig = self.gp.tile([128, 6 * BL], F32, tag=f"sig_{dr}", name=f"sig_{dr}")
                nc.scalar.activation(sig[:], g[:, :6 * BL], AF.Sigmoid)
                tg = self.gp.tile([128, 2 * BL], F32, tag=f"tg_{dr}", name=f"tg_{dr}")
                nc.scalar.activation(tg[:], g[:, 6 * BL:], AF.Tanh)

                c = self.c_sb[dr][:]
                t1 = self.gp.tile([128, 2 * BL], F32, tag=f"t1_{dr}", name=f"t1_{dr}")
                nc.vector.tensor_mul(t1[:], sig[:, :2 * BL], tg[:])        # i*g
                t2 = self.gp.tile([128, 2 * BL], F32, tag=f"t2_{dr}", name=f"t2_{dr}")
                nc.vector.tensor_mul(t2[:], sig[:, 2 * BL:4 * BL], c)      # f*c
                nc.vector.tensor_add(c, t1[:], t2[:])                      # c_n
                tcv = self.gp.tile([128, 2 * BL], F32, tag=f"tc_{dr}", name=f"tc_{dr}")
                nc.scalar.activation(tcv[:], c, AF.Tanh)

                hn = hist[:, :, _ds(texpr, 1), :]
                if not gated:
                    nc.vector.tensor_mul(hn, sig[:, 4 * BL:6 * BL], tcv[:])  # o*tanh(c)
                else:
                    ho = self.gp.tile([128, 2 * BL], F32, tag=f"ho_{dr}", name=f"ho_{dr}")
                    nc.vector.tensor_mul(ho[:], sig[:, 4 * BL:6 * BL], tcv[:])
                    u = self.gp.tile([128, 2 * BL], F32, tag=f"u_{dr}", name=f"u_{dr}")
                    cfs = cfb[dr][:, sx, :]
                    cf2 = bass.AP(
                        tensor=cfs.tensor,
                        offset=cfs.offset,
                        ap=[list(cfs.ap[0]), [0, 2], list(cfs.ap[1])],
                    )
                    nc.vector.tensor_mul(u[:], cf2, hprev)                 # cf*h_prev
                    nc.vector.tensor_add(hn, ho[:], u[:])

    # ---------------- coefficient bulk compute ----------------
    def coeff_bulk(self):
        nc = self.nc
        T = self.T
        total = T * BL
        NB = max(total // 512, 1)
        n = min(512, total)
        for dr in ("f", "b"):
            hflat = self.hist[("d", dr)][:].rearrange("p k t b -> p k (t b)")
            cfl = self.cf[dr][:].rearrange("one t b -> one (t b)")
            for nb in range(NB):
                pc = self.psum_c.tile([1, n], F32, tag="pc", name="coef")
                for kc in range(2):
                    nc.tensor.matmul(
                        pc[:],
                        lhsT=self.cwT[dr][:, kc],
                        rhs=hflat[:, kc, nb * n:(nb + 1) * n],
                        start=(kc == 0), stop=(kc == 1),
                    )
                # sigmoid(. + cb)
                nc.scalar.activation(
                    cfl[:, nb * n:(nb + 1) * n], pc[:], AF.Sigmoid,
                    bias=self.cb[dr][:],
                )
            nc.vector.tensor_scalar_mul(cfl, cfl, DF)

    # ---------------- output ----------------
    def write_out(self):
        nc = self.nc
        T = self.T
        TB = min(128, T)
        for dr_i, dr in enumerate(("f", "b")):
            hist = self.hist[("l1", dr)]
            for k in range(2):
                for b in range(BL):
                    for tb in range(T // TB):
                        ps = self.psum_x.tile([TB, 128], BF16, tag="px", name="tr")
                        nc.tensor.transpose(
                            ps[:], hist[:, k, tb * TB:(tb + 1) * TB, b],
                            self.ident[:])
                        st = self.outp.tile([TB, 128], F32, tag="st", name="st")
                        nc.vector.tensor_copy(st[:], ps[:])
                        nc.sync.dma_start(
                            self.d_out[b, tb * TB:(tb + 1) * TB,
                                       dr_i * 256 + k * 128:dr_i * 256 + (k + 1) * 128],
                            st[:])


# ================= host-side program cache =================
_PROG_CACHE = {}


def _build_program(T=T_FULL, use_for_i=True, debug_outs=False):
    key = (T, use_for_i, debug_outs)
    if key in _PROG_CACHE:
        return _PROG_CACHE[key]
    b = _Builder(T, use_for_i=use_for_i, debug_outs=debug_outs)
    b.declare_io()
    with tile.TileContext(b.nc) as tc:
        with ExitStack() as ctx:
            b.build(ctx, tc)
    b.nc.compile()
    _PROG_CACHE[key] = b
    return b


def _build_null_program():
    """Same I/O signature, trivial compute — measures dispatch/RTT floor."""
    key = ("null",)
    if key in _PROG_CACHE:
        return _PROG_CACHE[key]
    b = _Builder(T_FULL, use_for_i=True, debug_outs=False)
    b.declare_io()
    nc = b.nc
    with tile.TileContext(nc) as tc:
        with ExitStack() as ctx:
            pool = ctx.enter_context(tc.tile_pool(name="p", bufs=2))
            t = pool.tile([128, 512], F32, name="t")
            nc.sync.dma_start(t[:], b.d_out[0, 0:128, :])
            nc.vector.tensor_scalar_mul(t[:], t[:], 1.0)
            nc.sync.dma_start(b.d_out[0, 0:128, :], t[:])
    nc.compile()
    _PROG_CACHE[key] = b
    return b


# ================= host-side data prep =================
def _prep_weights(kw):
    """Permute/transpose/cast weights; returns name->array for DRAM inputs."""
    out = {}
    for ph, pre, nk in (("d", "d", 2), ("l0", "l0", 2), ("l1", "l1", 4)):
        for dr in ("f", "b"):
            Wih = np.asarray(kw[f"{pre}Wih_{dr}"], np.float32)[_PERM]     # [4H, in]
            Whh = np.asarray(kw[f"{pre}Whh_{dr}"], np.float32)[_PERM]     # [4H, H]
            bias = (np.asarray(kw[f"{pre}bih_{dr}"], np.float32)
                    + np.asarray(kw[f"{pre}bhh_{dr}"], np.float32))[_PERM]
            out[f"{ph}wih_{dr}"] = np.ascontiguousarray(Wih.T).astype(ml_dtypes.bfloat16)
            out[f"{ph}whh_{dr}"] = np.ascontiguousarray(Whh.T).astype(ml_dtypes.bfloat16)
            out[f"{ph}bias_{dr}"] = np.ascontiguousarray(
                bias.reshape(NCH, 128).T).astype(np.float32)
    for dr in ("f", "b"):
        out[f"cwT_{dr}"] = np.ascontiguousarray(
            np.asarray(kw[f"cw_{dr}"], np.float32).reshape(1, H).T).astype(ml_dtypes.bfloat16)
        out[f"cb_{dr}"] = np.asarray(kw[f"cb_{dr}"], np.float32).reshape(1, 1)
    out["ident"] = np.eye(128, dtype=ml_dtypes.bfloat16)
    out["ones"] = np.ones((1, 128), dtype=ml_dtypes.bfloat16)
    return out


def _in_maps(inputs_np, T):
    x = np.asarray(inputs_np["inputs"], np.float32)[:, :T]
    wmaps = _prep_weights(inputs_np)
    maps = []
    for c in range(NCORES):
        xs = x[c * BL:(c + 1) * BL]                                # [BL, T, I]
        xT = np.ascontiguousarray(xs.transpose(2, 1, 0)).astype(ml_dtypes.bfloat16)
        m = dict(wmaps)
        m["xT"] = xT
        maps.append(m)
    return maps


def _run(inputs_np, T=T_FULL, use_for_i=True, debug_outs=False, trace=False,
         trace_kwargs=None):
    prog = _build_program(T, use_for_i, debug_outs)
    in_maps = _in_maps(inputs_np, T)
    res = bass_utils.run_bass_kernel_spmd(
        prog.nc, in_maps, list(range(NCORES)), trace=trace,
        **(trace_kwargs or {}))
    outs = np.concatenate([res.results[c]["out"] for c in range(NCORES)], axis=0)
    return outs, res


def kernel(**inputs):
    out, _ = _run(inputs)
    return out.astype(np.float32)


# revision 15
# speedup vs baseline: 4.0268x; 4.0268x over previous
"""Trainium2 Bass kernel for nn_DTRN: 2-layer bidirectional discount-gated LSTM
with a bidirectional-LSTM discount-coefficient generator.

Sharding: data-parallel over batch, B=16 -> 2 per core across 8 cores (SPMD,
no collectives). Per core, three sequential scan phases (d -> l0 -> l1), each
running the forward and backward time scans interleaved.

Layouts (per core, BL = local batch = 2):
  - Gates in PSUM chunk-layout [128, 8*BL]: 4H=1024 split into 8 chunks of 128
    rows; host permutes gate order to (i,f,o,g) so one sigmoid covers a
    contiguous 6*BL block and tanh the trailing 2*BL block.
  - Recurrent matmul is weight-stationary bf16: 16 [128x128]x[128,BL] matmuls
    per step accumulating onto the precomputed input projection (injected into
    PSUM by an identity matmul).
  - Input projections computed in bulk per 64-step block from a transposed
    SBUF-resident copy of the inputs; bias folded in during the PSUM->SBUF
    copy on ScalarE (which also casts to bf16).
  - h histories stay in SBUF transposed ([128, k, T, BL] bf16): they feed the
    next step's recurrent matmul, the next layer's bulk projections, the
    coefficient projections, and the final PE-transpose output path.
  - Discount coefficients are computed in bulk between phases (matvec +
    sigmoid), then broadcast across partitions per block via a rank-1 matmul
    against a ones vector.
"""

import sys

sys.path.insert(0, "/opt/trn_rl_repo")

from contextlib import ExitStack

import numpy as np
import ml_dtypes

import concourse.bass as bass
import concourse.tile as tile
from concourse import bacc, mybir
from concourse import bass_utils

F32 = mybir.dt.float32
BF16 = mybir.dt.bfloat16
AF = mybir.ActivationFunctionType
DF = 0.9

B, T_FULL, I, H = 16, 2048, 256, 256
NCORES = 8
BL = B // NCORES  # local batch per core
G4 = 4 * H  # 1024
NCH = G4 // 128  # 8 gate chunks
S = 64  # scan steps per block

# --- optimization switches (host weight prep must match the kernel) ---
SIG2 = True          # tanh(x) = 2*sigmoid(2x)-1 with g-rows pre-doubled: 2 ACT/step
GPS_B_CHAIN = True   # run backward chain's elementwise on GPSIMD

# gate-permutation: reorder (i,f,g,o) -> (i,f,o,g) so the sigmoid block is
# contiguous (chunks 0..5) and tanh covers chunks 6..7
_PERM = np.r_[0:2 * H, 3 * H:4 * H, 2 * H:3 * H]


def _ds(e, n):
    return bass.ds(e, n)


class _Builder:
    def __init__(self, T, use_for_i=True, debug_outs=False,
                 phases=("d", "l0", "l1")):
        assert T % S == 0
        self.T = T
        self.use_for_i = use_for_i
        self.debug_outs = debug_outs
        self.phases = tuple(phases)
        self.nc = bacc.Bacc("TRN2", target_bir_lowering=False, debug=False)

    # ---------------- DRAM I/O ----------------
    def declare_io(self):
        nc = self.nc
        T = self.T
        if "d" in self.phases or "l0" in self.phases:
            self.d_xT = nc.dram_tensor("xT", [2 * 128, T, BL], BF16, kind="ExternalInput").ap()
        self.d_cf_io = {}
        if self.phases[0] != "d":
            for dr in ("f", "b"):
                self.d_cf_io[dr] = nc.dram_tensor(
                    f"cfio_{dr}", [1, T, BL], BF16, kind="ExternalInput").ap()
        elif "l1" not in self.phases:
            for dr in ("f", "b"):
                self.d_cf_io[dr] = nc.dram_tensor(
                    f"cfio_{dr}", [1, T, BL], BF16, kind="ExternalOutput").ap()
        self.d_l0hist_io = {}
        if "l1" in self.phases and "l0" not in self.phases:
            for dr in ("f", "b"):
                self.d_l0hist_io[dr] = nc.dram_tensor(
                    f"l0hio_{dr}", [128, 2, T, BL], BF16, kind="ExternalInput").ap()
        elif "l0" in self.phases and "l1" not in self.phases:
            for dr in ("f", "b"):
                self.d_l0hist_io[dr] = nc.dram_tensor(
                    f"l0hio_{dr}", [128, 2, T, BL], BF16, kind="ExternalOutput").ap()
        self.d_w = {}
        for ph, nk in (("d", 2), ("l0", 2), ("l1", 4)):
            if ph not in self.phases:
                continue
            for dr in ("f", "b"):
                self.d_w[f"{ph}wih_{dr}"] = nc.dram_tensor(
                    f"{ph}wih_{dr}", [nk * 128, G4], BF16, kind="ExternalInput").ap()
                self.d_w[f"{ph}whh_{dr}"] = nc.dram_tensor(
                    f"{ph}whh_{dr}", [2 * 128, G4], BF16, kind="ExternalInput").ap()
                self.d_w[f"{ph}bias_{dr}"] = nc.dram_tensor(
                    f"{ph}bias_{dr}", [128, NCH], F32, kind="ExternalInput").ap()
        if "d" in self.phases:
            for dr in ("f", "b"):
                self.d_w[f"cwT_{dr}"] = nc.dram_tensor(
                    f"cwT_{dr}", [2 * 128, 1], BF16, kind="ExternalInput").ap()
                self.d_w[f"cb_{dr}"] = nc.dram_tensor(
                    f"cb_{dr}", [1, 1], F32, kind="ExternalInput").ap()
        self.d_ident = nc.dram_tensor("ident", [128, 128], BF16, kind="ExternalInput").ap()
        self.d_ones = nc.dram_tensor("ones", [1, 128], BF16, kind="ExternalInput").ap()
        if "l1" in self.phases:
            self.d_out = nc.dram_tensor("out", [BL, T, 2 * H], F32, kind="ExternalOutput").ap()
        self.d_dbg = {}
        if self.debug_outs:
            for nm in ("hist_d_f", "hist_d_b", "hist_l0_f", "hist_l0_b"):
                self.d_dbg[nm] = nc.dram_tensor(
                    "dbg_" + nm, [128, 2, self.T, BL], BF16, kind="ExternalOutput").ap()
            for nm in ("cf_f", "cf_b"):
                self.d_dbg[nm] = nc.dram_tensor(
                    "dbg_" + nm, [1, self.T, BL], BF16, kind="ExternalOutput").ap()

    # ---------------- build ----------------
    def build(self, ctx: ExitStack, tc: tile.TileContext):
        nc = self.nc
        T = self.T
        self.tc = tc

        persist = ctx.enter_context(tc.tile_pool(name="persist", bufs=1))
        wpool = ctx.enter_context(tc.tile_pool(name="weights", bufs=1))
        self.psum_g = ctx.enter_context(tc.tile_pool(name="psum_g", bufs=4, space="PSUM"))
        self.psum_x = ctx.enter_context(tc.tile_pool(name="psum_x", bufs=2, space="PSUM"))
        self.psum_c = ctx.enter_context(tc.tile_pool(name="psum_c", bufs=2, space="PSUM"))
        self.xp_pool = ctx.enter_context(tc.tile_pool(name="xp_stage", bufs=2))
        self.cfb_pool = ctx.enter_context(tc.tile_pool(name="cfb", bufs=2))
        self.gp = ctx.enter_context(tc.tile_pool(name="gates", bufs=4))
        self.outp = ctx.enter_context(tc.tile_pool(name="outstage", bufs=4))

        # --- constants & inputs resident in SBUF ---
        self.ident = persist.tile([128, 128], BF16, tag="ident", name="ident")
        nc.sync.dma_start(self.ident[:], self.d_ident)
        self.ones = persist.tile([1, 128], BF16, tag="ones", name="ones")
        nc.sync.dma_start(self.ones[:], self.d_ones)
        self.zrow = persist.tile([128, 2, BL], BF16, tag="zrow", name="zrow")
        nc.vector.memset(self.zrow[:], 0.0)

        if "d" in self.phases or "l0" in self.phases:
            self.xT = persist.tile([128, 2, T, BL], BF16, tag="xT", name="xT")
            for k in range(2):
                nc.sync.dma_start(self.xT[:, k], self.d_xT[k * 128:(k + 1) * 128])

        # histories (only for phases present; l1 needs l0's)
        self.hist = {}
        for ph in ("d", "l0", "l1"):
            if ph not in self.phases and not (ph == "l0" and "l1" in self.phases):
                continue
            for dr in ("f", "b"):
                self.hist[(ph, dr)] = persist.tile(
                    [128, 2, T, BL], BF16, tag=f"hist_{ph}_{dr}", name=f"hist_{ph}_{dr}")
        if "l1" in self.phases and "l0" not in self.phases:
            for dr in ("f", "b"):
                nc.sync.dma_start(self.hist[("l0", dr)][:], self.d_l0hist_io[dr])

        # coefficient buffers (written after d phase)
        self.cf = {dr: persist.tile([1, T, BL], BF16, tag=f"cf_{dr}", name=f"cf_{dr}")
                   for dr in ("f", "b")}

        # c state
        self.c_sb = {dr: persist.tile([128, 2, BL], F32, tag=f"c_{dr}", name=f"c_{dr}")
                     for dr in ("f", "b")}

        # coefficient weights / preloaded coefficients
        self.cwT = {}
        self.cb = {}
        if "d" in self.phases:
            for dr in ("f", "b"):
                t = persist.tile([128, 2, 1], BF16, tag=f"cwT_{dr}", name=f"cwT_{dr}")
                for k in range(2):
                    nc.sync.dma_start(t[:, k], self.d_w[f"cwT_{dr}"][k * 128:(k + 1) * 128])
                self.cwT[dr] = t
                tb = persist.tile([1, 1], F32, tag=f"cb_{dr}", name=f"cb_{dr}")
                nc.sync.dma_start(tb[:], self.d_w[f"cb_{dr}"])
                self.cb[dr] = tb
        elif self.phases[0] != "d":
            for dr in ("f", "b"):
                nc.sync.dma_start(self.cf[dr][:], self.d_cf_io[dr])

        # --- phases ---
        for ph, nk in (("d", 2), ("l0", 2), ("l1", 4)):
            if ph not in self.phases:
                continue
            wih, whh, bias = {}, {}, {}
            for dr in ("f", "b"):
                w1 = wpool.tile([128, 4, G4], BF16, tag=f"wih_{dr}", name=f"{ph}wih_{dr}")
                for k in range(nk):
                    nc.sync.dma_start(w1[:, k], self.d_w[f"{ph}wih_{dr}"][k * 128:(k + 1) * 128])
                wih[dr] = w1
                w2 = wpool.tile([128, 2, G4], BF16, tag=f"whh_{dr}", name=f"{ph}whh_{dr}")
                for k in range(2):
                    nc.sync.dma_start(w2[:, k], self.d_w[f"{ph}whh_{dr}"][k * 128:(k + 1) * 128])
                whh[dr] = w2
                bt = wpool.tile([128, NCH], F32, tag=f"bias_{dr}", name=f"{ph}bias_{dr}")
                nc.sync.dma_start(bt[:], self.d_w[f"{ph}bias_{dr}"])
                bias[dr] = bt
                nc.vector.memset(self.c_sb[dr][:], 0.0)
            self.phase(ph, nk, wih, whh, bias)
            if ph == "d":
                self.coeff_bulk()

        if "l1" in self.phases:
            self.write_out()
        if "d" in self.phases and "l1" not in self.phases:
            for dr in ("f", "b"):
                nc.sync.dma_start(self.d_cf_io[dr], self.cf[dr][:])
        if "l0" in self.phases and "l1" not in self.phases:
            for dr in ("f", "b"):
                nc.sync.dma_start(self.d_l0hist_io[dr], self.hist[("l0", dr)][:])
        for nm, ap in self.d_dbg.items():
            if nm.startswith("hist"):
                _, p2, d2 = nm.split("_")
                nc.sync.dma_start(ap, self.hist[(p2, d2)][:])
            else:
                dr = nm.split("_")[1]
                nc.sync.dma_start(ap, self.cf[dr][:])

    # ---------------- xp source ----------------
    def xp_rhs(self, ph, kc, texpr, n):
        """rhs [128, n, BL] (t-ascending) for bulk input projection, chunk kc."""
        if ph in ("d", "l0"):
            return self.xT[:, kc, _ds(texpr, n), :]
        src = self.hist[("l0", "f")] if kc < 2 else self.hist[("l0", "b")]
        return src[:, kc % 2, _ds(texpr, n), :]

    # ---------------- one phase ----------------
    def phase(self, ph, nk, wih, whh, bias):
        nblk = self.T // S

        self.block(ph, nk, wih, whh, bias, 0, True)
        if nblk > 1:
            if self.use_for_i:
                with self.tc.For_i(
                        1, nblk, 1,
                        hint_engines=tuple(mybir.ALL_ENGINES),
                        staggered_reset=True) as i:
                    self.block(ph, nk, wih, whh, bias, i, False)
            else:
                for i in range(1, nblk):
                    self.block(ph, nk, wih, whh, bias, i, False)

    # ---------------- one 64-step block ----------------
    def block(self, ph, nk, wih, whh, bias, i, first):
        nc = self.nc
        T = self.T
        t0f = i * S            # forward block start (ascending)
        t0b = (T - S) - i * S  # backward block covers [t0b, t0b+S), consumed descending

        gated = ph != "d"
        xp = {}
        cfb = {}
        for dr, t0 in (("f", t0f), ("b", t0b)):
            # ---- bulk input projection for this block ----
            stage = self.xp_pool.tile([128, S, NCH, BL], BF16, tag=f"xp_{dr}",
                                      name=f"xp_{dr}")
            for m in range(NCH):
                px = self.psum_x.tile([128, S * BL], F32, tag="px", name="px")
                for kc in range(nk):
                    nc.tensor.matmul(
                        px[:],
                        lhsT=wih[dr][:, kc, m * 128:(m + 1) * 128],
                        rhs=self.xp_rhs(ph, kc, t0, S),
                        start=(kc == 0),
                        stop=(kc == nk - 1),
                    )
                # psum -> staging with bias add + bf16 cast (ScalarE)
                nc.scalar.activation(
                    stage[:, :, m, :],
                    px[:],
                    AF.Identity,
                    bias=bias[dr][:, m:m + 1],
                )
            xp[dr] = stage
            if gated:
                # ---- broadcast coefficients across partitions for this block ----
                pc = self.psum_c.tile([128, S * BL], F32, tag="pc", name="pc")
                nc.tensor.matmul(
                    pc[:],
                    lhsT=self.ones[:],
                    rhs=self.cf[dr][:, _ds(t0, S), :],
                    start=True,
                    stop=True,
                )
                cfv = self.cfb_pool.tile([128, S, BL], BF16, tag=f"cfb_{dr}",
                                         name=f"cfb_{dr}")
                nc.vector.tensor_copy(cfv[:], pc[:])
                cfb[dr] = cfv

        # ---- the 64 scan steps ----
        for s in range(S):
            for dr in ("f", "b"):
                if dr == "f":
                    sx = s
                    texpr = t0f + s
                else:
                    sx = S - 1 - s
                    texpr = t0b + sx
                hist = self.hist[(ph, dr)]
                at_start = first and s == 0
                if at_start:
                    hprev = self.zrow[:]
                else:
                    tprev = (t0f + s - 1) if dr == "f" else (t0b + sx + 1)
                    hprev = hist[:, :, _ds(tprev, 1), :]

                g = self.psum_g.tile([128, NCH * BL], F32, tag="g", name="g")
                nc.tensor.matmul(
                    g[:], lhsT=self.ident[:],
                    rhs=xp[dr][:, sx],
                    start=True, stop=False, skip_group_check=True,
                )
                for kc in range(2):
                    hk = self.zrow[:, kc] if at_start else hprev[:, kc]
                    for m in range(NCH):
                        nc.tensor.matmul(
                            g[:, m * BL:(m + 1) * BL],
                            lhsT=whh[dr][:, kc, m * 128:(m + 1) * 128],
                            rhs=hk,
                            start=False, stop=(kc == 1 and m == NCH - 1),
                            skip_group_check=True,
                        )

                # TT ops for the backward chain go to GPSIMD to offload DVE;
                # scalar_tensor_tensor is only legal on DVE (walrus NCC_IXCG966)
                ve = nc.gpsimd if (GPS_B_CHAIN and dr == "b") else nc.vector
                c = self.c_sb[dr][:]
                hn = hist[:, :, _ds(texpr, 1), :]
                cf2 = None
                if gated:
                    cfs = cfb[dr][:, sx, :]
                    cf2 = bass.AP(
                        tensor=cfs.tensor,
                        offset=cfs.offset,
                        ap=[list(cfs.ap[0]), [0, 2], list(cfs.ap[1])],
                    )
                MUL = mybir.AluOpType.mult

                if SIG2:
                    # one sigmoid covers i,f,o and 2g (g-rows pre-doubled)
                    sig = self.gp.tile([128, 8 * BL], F32, tag=f"sig_{dr}",
                                       name=f"sig_{dr}")
                    nc.scalar.activation(sig[:], g[:], AF.Sigmoid)
                    si, sf, so = sig[:, :2 * BL], sig[:, 2 * BL:4 * BL], sig[:, 4 * BL:6 * BL]
                    s2g = sig[:, 6 * BL:]
                    a1 = self.gp.tile([128, 2 * BL], F32, tag=f"t1_{dr}", name=f"a1_{dr}")
                    nc.vector.scalar_tensor_tensor(a1[:], s2g, 2.0, si, MUL, MUL)  # 2*si*s2g
                    t1 = self.gp.tile([128, 2 * BL], F32, tag=f"tg_{dr}", name=f"t1_{dr}")
                    ve.tensor_sub(t1[:], a1[:], si)                  # si*tanh(g)
                    a3 = self.gp.tile([128, 2 * BL], F32, tag=f"t2_{dr}", name=f"a3_{dr}")
                    ve.tensor_mul(a3[:], sf, c)                      # f*c
                    ve.tensor_add(c, t1[:], a3[:])                   # c_n
                    s2c = self.gp.tile([128, 2 * BL], F32, tag=f"tc_{dr}", name=f"s2c_{dr}")
                    nc.scalar.activation(s2c[:], c, AF.Sigmoid, scale=2.0)
                    b1 = self.gp.tile([128, 2 * BL], F32, tag=f"ho_{dr}", name=f"b1_{dr}")
                    nc.vector.scalar_tensor_tensor(b1[:], s2c[:], 2.0, so, MUL, MUL)  # 2*so*s2c
                    if not gated:
                        ve.tensor_sub(hn, b1[:], so)                 # o*tanh(c)
                    else:
                        b2 = self.gp.tile([128, 2 * BL], F32, tag=f"b2_{dr}", name=f"b2_{dr}")
                        ve.tensor_sub(b2[:], b1[:], so)
                        u = self.gp.tile([128, 2 * BL], F32, tag=f"u_{dr}", name=f"u_{dr}")
                        ve.tensor_mul(u[:], cf2, hprev)              # cf*h_prev
                        ve.tensor_add(hn, b2[:], u[:])
                else:
                    sig = self.gp.tile([128, 6 * BL], F32, tag=f"sig_{dr}", name=f"sig_{dr}")
                    nc.scalar.activation(sig[:], g[:, :6 * BL], AF.Sigmoid)
                    tg = self.gp.tile([128, 2 * BL], F32, tag=f"tg_{dr}", name=f"tg_{dr}")
                    nc.scalar.activation(tg[:], g[:, 6 * BL:], AF.Tanh)

                    t1 = self.gp.tile([128, 2 * BL], F32, tag=f"t1_{dr}", name=f"t1_{dr}")
                    ve.tensor_mul(t1[:], sig[:, :2 * BL], tg[:])        # i*g
                    t2 = self.gp.tile([128, 2 * BL], F32, tag=f"t2_{dr}", name=f"t2_{dr}")
                    ve.tensor_mul(t2[:], sig[:, 2 * BL:4 * BL], c)      # f*c
                    ve.tensor_add(c, t1[:], t2[:])                      # c_n
                    tcv = self.gp.tile([128, 2 * BL], F32, tag=f"tc_{dr}", name=f"tc_{dr}")
                    nc.scalar.activation(tcv[:], c, AF.Tanh)

                    if not gated:
                        ve.tensor_mul(hn, sig[:, 4 * BL:6 * BL], tcv[:])  # o*tanh(c)
                    else:
                        ho = self.gp.tile([128, 2 * BL], F32, tag=f"ho_{dr}", name=f"ho_{dr}")
                        ve.tensor_mul(ho[:], sig[:, 4 * BL:6 * BL], tcv[:])
                        u = self.gp.tile([128, 2 * BL], F32, tag=f"u_{dr}", name=f"u_{dr}")
                        ve.tensor_mul(u[:], cf2, hprev)                 # cf*h_prev
                        ve.tensor_add(hn, ho[:], u[:])

    # ---------------- coefficient bulk compute ----------------
    def coeff_bulk(self):
        nc = self.nc
        T = self.T
        total = T * BL
        NB = max(total // 512, 1)
        n = min(512, total)
        for dr in ("f", "b"):
            hflat = self.hist[("d", dr)][:].rearrange("p k t b -> p k (t b)")
            cfl = self.cf[dr][:].rearrange("one t b -> one (t b)")
            for nb in range(NB):
                pc = self.psum_c.tile([1, n], F32, tag="pc", name="coef")
                for kc in range(2):
                    nc.tensor.matmul(
                        pc[:],
                        lhsT=self.cwT[dr][:, kc],
                        rhs=hflat[:, kc, nb * n:(nb + 1) * n],
                        start=(kc == 0), stop=(kc == 1),
                    )
                # sigmoid(. + cb)
                nc.scalar.activation(
                    cfl[:, nb * n:(nb + 1) * n], pc[:], AF.Sigmoid,
                    bias=self.cb[dr][:],
                )
            nc.vector.tensor_scalar_mul(cfl, cfl, DF)

    # ---------------- output ----------------
    def write_out(self):
        nc = self.nc
        T = self.T
        TB = min(128, T)
        for dr_i, dr in enumerate(("f", "b")):
            hist = self.hist[("l1", dr)]
            for k in range(2):
                for b in range(BL):
                    for tb in range(T // TB):
                        ps = self.psum_x.tile([TB, 128], BF16, tag="px", name="tr")
                        nc.tensor.transpose(
                            ps[:], hist[:, k, tb * TB:(tb + 1) * TB, b],
                            self.ident[:])
                        st = self.outp.tile([TB, 128], F32, tag="st", name="st")
                        nc.vector.tensor_copy(st[:], ps[:])
                        nc.sync.dma_start(
                            self.d_out[b, tb * TB:(tb + 1) * TB,
                                       dr_i * 256 + k * 128:dr_i * 256 + (k + 1) * 128],
                            st[:])


# ================= host-side program cache =================
_PROG_CACHE = {}


def _build_program(T=T_FULL, use_for_i=False, debug_outs=False,
                   phases=("d", "l0", "l1")):
    key = (T, use_for_i, debug_outs, tuple(phases))
    if key in _PROG_CACHE:
        return _PROG_CACHE[key]
    b = _Builder(T, use_for_i=use_for_i, debug_outs=debug_outs, phases=phases)
    b.declare_io()
    with tile.TileContext(b.nc) as tc:
        with ExitStack() as ctx:
            b.build(ctx, tc)
    b.nc.compile()
    _PROG_CACHE[key] = b
    return b


def _build_null_program():
    """Same I/O signature, trivial compute — measures dispatch/RTT floor."""
    key = ("null",)
    if key in _PROG_CACHE:
        return _PROG_CACHE[key]
    b = _Builder(T_FULL, use_for_i=True, debug_outs=False)
    b.declare_io()
    nc = b.nc
    with tile.TileContext(nc) as tc:
        with ExitStack() as ctx:
            pool = ctx.enter_context(tc.tile_pool(name="p", bufs=2))
            t = pool.tile([128, 512], F32, name="t")
            nc.sync.dma_start(t[:], b.d_out[0, 0:128, :])
            nc.vector.tensor_scalar_mul(t[:], t[:], 1.0)
            nc.sync.dma_start(b.d_out[0, 0:128, :], t[:])
    nc.compile()
    _PROG_CACHE[key] = b
    return b


# ================= host-side data prep =================
def _prep_weights(kw):
    """Permute/transpose/cast weights; returns name->array for DRAM inputs."""
    out = {}
    for ph, pre, nk in (("d", "d", 2), ("l0", "l0", 2), ("l1", "l1", 4)):
        for dr in ("f", "b"):
            Wih = np.asarray(kw[f"{pre}Wih_{dr}"], np.float32)[_PERM]     # [4H, in]
            Whh = np.asarray(kw[f"{pre}Whh_{dr}"], np.float32)[_PERM]     # [4H, H]
            bias = (np.asarray(kw[f"{pre}bih_{dr}"], np.float32)
                    + np.asarray(kw[f"{pre}bhh_{dr}"], np.float32))[_PERM]
            if SIG2:
                # g-gate rows (last two chunks after permutation) doubled so a
                # single sigmoid yields sigmoid(2g); tanh(g) = 2*sigmoid(2g)-1
                Wih = Wih.copy(); Whh = Whh.copy(); bias = bias.copy()
                Wih[6 * 128:] *= 2.0
                Whh[6 * 128:] *= 2.0
                bias[6 * 128:] *= 2.0
            out[f"{ph}wih_{dr}"] = np.ascontiguousarray(Wih.T).astype(ml_dtypes.bfloat16)
            out[f"{ph}whh_{dr}"] = np.ascontiguousarray(Whh.T).astype(ml_dtypes.bfloat16)
            out[f"{ph}bias_{dr}"] = np.ascontiguousarray(
                bias.reshape(NCH, 128).T).astype(np.float32)
    for dr in ("f", "b"):
        out[f"cwT_{dr}"] = np.ascontiguousarray(
            np.asarray(kw[f"cw_{dr}"], np.float32).reshape(1, H).T).astype(ml_dtypes.bfloat16)
        out[f"cb_{dr}"] = np.asarray(kw[f"cb_{dr}"], np.float32).reshape(1, 1)
    out["ident"] = np.eye(128, dtype=ml_dtypes.bfloat16)
    out["ones"] = np.ones((1, 128), dtype=ml_dtypes.bfloat16)
    return out


def _in_maps(inputs_np, T):
    x = np.asarray(inputs_np["inputs"], np.float32)[:, :T]
    wmaps = _prep_weights(inputs_np)
    maps = []
    for c in range(NCORES):
        xs = x[c * BL:(c + 1) * BL]                                # [BL, T, I]
        xT = np.ascontiguousarray(xs.transpose(2, 1, 0)).astype(ml_dtypes.bfloat16)
        m = dict(wmaps)
        m["xT"] = xT
        maps.append(m)
    return maps


SPLIT_PHASES = None  # e.g. (("d",), ("l0",), ("l1",)) or None for single program


def _run(inputs_np, T=T_FULL, use_for_i=False, debug_outs=False, trace=False,
         trace_kwargs=None):
    in_maps = _in_maps(inputs_np, T)
    if SPLIT_PHASES is None:
        prog = _build_program(T, use_for_i, debug_outs)
        res = bass_utils.run_bass_kernel_spmd(
            prog.nc, in_maps, list(range(NCORES)), trace=trace,
            **(trace_kwargs or {}))
        outs = np.concatenate([res.results[c]["out"] for c in range(NCORES)], axis=0)
        return outs, res
    res = None
    carry = [dict(m) for m in in_maps]
    for phs in SPLIT_PHASES:
        prog = _build_program(T, use_for_i, debug_outs, phases=phs)
        res = bass_utils.run_bass_kernel_spmd(
            prog.nc, carry, list(range(NCORES)), trace=trace,
            **(trace_kwargs or {}))
        for c in range(NCORES):
            carry[c].update(res.results[c])
    outs = np.concatenate([res.results[c]["out"] for c in range(NCORES)], axis=0)
    return outs, res


def kernel(**inputs):
    out, _ = _run(inputs)
    return out.astype(np.float32)
